# revision 1
# baseline (speedup 1.0000x reference)
"""Trainium2 Bass kernel for nn_Attention (dense_transformer).

Sharding: 8 cores = 4 batches x 2 query-halves (800 query positions each).
No collectives: each core computes its half's full attention (keys/values over
all 1600 positions), the depthwise 3x3 branch for its rows (with halo), and the
final 1x1 projection for its positions.

Math notes (host-folded):
  - All BN affine transforms folded into conv weights + bias vectors.
  - K's BN bias is dropped (adds a per-query constant to logits -> softmax
    invariant); Q's bias is applied on-chip.
  - V's BN bias is applied analytically: softmax rows sum to 1, so the
    attention branch gets +bias_v per channel; the dw branch's bias (incl.
    zero-padding edge effects) is a rank-10 correction matmul from a
    host-built mask.
  - Softmax skips max-subtraction (logits ~ [-8, 8], exp is safe in f32).
  - attn^T orientation: logits[m, n] tiles with keys m on partitions; the
    softmax denominator comes from a ones-column appended to v^T (col-tiled
    matmul), so no partition reductions are needed.
"""
import sys

sys.path.insert(0, "/opt/trn_rl_repo")

import numpy as np
import ml_dtypes

BF16 = ml_dtypes.bfloat16
CH = 256
NH = 4
DH = 64
DK = 32
QKV_C = 512
EPS = 1e-3
B = 4
H = W = 40
HW = 1600
NQ = 800          # query positions per core
NCORES = 8
MB = 13           # m blocks: 12*128 + 64
SCALE = DK ** -0.5

_COMPILED = None
LAST_EXEC_NS = None
LAST_RESULTS = None


def _mbs(i):
    """size of m-block i"""
    return 64 if i == MB - 1 else 128


# n-chunks of the 800 query positions (psum-bank aligned)
NCHUNKS = ((0, 512), (512, 800))



def _emit_av(nc, pavs, vT, i, et, nw, NH, DH, MB):
    mb = 64 if i == MB - 1 else 128
    for h in range(NH):
        # single M=65 matmul: rows 0:64 = A@V^T, row 64 = softmax denominator
        nc.tensor.matmul(pavs[h][0:65, 0:nw], vT[:mb, i, h, 0:DH + 1],
                         et[:mb, h, 0:nw], start=(i == 0), stop=(i == MB - 1),
                         skip_group_check=True)

def build_program():
    import os
    import concourse.bass as bass
    import concourse.bacc as bacc
    import concourse.tile as tile
    from concourse import mybir

    max_phase = int(os.environ.get("KERNEL_PHASES", "5"))
    skip_set = set(filter(None, os.environ.get("KERNEL_SKIP", "").split(",")))

    f32 = mybir.dt.float32
    bf16 = mybir.dt.bfloat16

    nc = bacc.Bacc("TRN2", target_bir_lowering=False, debug=False,
                   enable_asserts=False)

    # ---- DRAM parameters (per-core data supplied via in_maps) ----
    xf_d = nc.dram_tensor("xf", [CH, HW], bf16, kind="ExternalInput")[:, :]
    xh_d = nc.dram_tensor("xh", [CH, 80], bf16, kind="ExternalInput")[:, :]
    wqkvT_d = nc.dram_tensor("wqkvT", [CH, QKV_C], bf16, kind="ExternalInput")[:, :]
    bq_d = nc.dram_tensor("bq", [128, 1], f32, kind="ExternalInput")[:, :]
    wdiag_d = nc.dram_tensor("wdiag", [128, 18 * 128], bf16, kind="ExternalInput")[:, :]
    wdwb_d = nc.dram_tensor("wdwb", [10, CH], bf16, kind="ExternalInput")[:, :]
    maug_d = nc.dram_tensor("maug", [10, NQ], bf16, kind="ExternalInput")[:, :]
    wprojT_d = nc.dram_tensor("wprojT", [CH, CH], bf16, kind="ExternalInput")[:, :]
    bproj_d = nc.dram_tensor("bproj", [CH, 1], f32, kind="ExternalInput")[:, :]
    out_d = nc.dram_tensor("out", [CH, NQ], f32, kind="ExternalOutput")[:, :]

    with tile.TileContext(nc) as tc:
        with tc.tile_pool(name="persist", bufs=1) as persist:
            # ---- load inputs ----
            xf = [persist.tile([128, HW], bf16, name=f"xf{j}") for j in range(2)]
            for j in range(2):
                nc.sync.dma_start(out=xf[j], in_=xf_d[128 * j:128 * (j + 1), :])
            xh = [persist.tile([128, 80], bf16, name=f"xh{j}") for j in range(2)]
            for j in range(2):
                nc.sync.dma_start(out=xh[j], in_=xh_d[128 * j:128 * (j + 1), :])
            wqkvT = [persist.tile([128, QKV_C], bf16, name=f"wqkvT{j}")
                     for j in range(2)]
            for j in range(2):
                nc.sync.dma_start(out=wqkvT[j],
                                  in_=wqkvT_d[128 * j:128 * (j + 1), :])
            bq = persist.tile([128, 1], f32, name="bq")
            nc.sync.dma_start(out=bq, in_=bq_d)
            wdiag = persist.tile([128, 18, 128], bf16, name="wdiag")
            nc.sync.dma_start(out=wdiag, in_=wdiag_d.rearrange(
                "p (t k) -> p t k", t=18))
            wdwb = persist.tile([10, CH], bf16, name="wdwb")
            nc.sync.dma_start(out=wdwb, in_=wdwb_d)
            maug = persist.tile([10, NQ], bf16, name="maug")
            nc.sync.dma_start(out=maug, in_=maug_d)
            wprojT = [persist.tile([128, CH], bf16, name=f"wprojT{j}")
                      for j in range(2)]
            for j in range(2):
                nc.sync.dma_start(out=wprojT[j],
                                  in_=wprojT_d[128 * j:128 * (j + 1), :])
            bproj = [persist.tile([128, 1], f32, name=f"bproj{j}")
                     for j in range(2)]
            for j in range(2):
                nc.sync.dma_start(out=bproj[j],
                                  in_=bproj_d[128 * j:128 * (j + 1), :])

            # persistent working tensors
            Q = persist.tile([128, NQ], bf16, name="Q")
            K = persist.tile([128, HW], bf16, name="K")
            vT = persist.tile([128, MB, NH, DH + 1], bf16, name="vT")
            vpad = [persist.tile([128, 22, 42], bf16, name=f"vpad{j}")
                    for j in range(2)]
            attn_out = [persist.tile([128, NQ], bf16, name=f"attn_out{j}")
                        for j in range(2)]
            dw_sb = [persist.tile([128, NQ], bf16, name=f"dw_sb{j}")
                     for j in range(2)]
            out_sb = [persist.tile([128, NQ], f32, name=f"out_sb{j}")
                      for j in range(2)]

            for j in range(2):
                nc.vector.memset(vpad[j], 0.0)
                nc.vector.memset(out_sb[j], 0.0)
                nc.vector.memset(dw_sb[j], 0.0)
                nc.vector.memset(attn_out[j], 0.0)
            onesrow = persist.tile([65, 64], f32, name="onesrow")
            nc.vector.memset(onesrow[64:65, :], 1.0)
            # dummy Exp so the ~2.7us ACT table load overlaps the initial
            # DMAs instead of stalling the first attention block
            actwarm = persist.tile([65, 1], f32, name="actwarm")
            nc.scalar.activation(actwarm[64:65, :], onesrow[64:65, 0:1],
                                 mybir.ActivationFunctionType.Exp)

            # ================= Phase 1: QKV conv (1x1) =================
            # wqkvT columns: [Q(128) | K(128) | V(256 = 4 heads x 64)]
            with tc.tile_pool(name="ps_qkv", bufs=4, space="PSUM") as psq, \
                 tc.tile_pool(name="ps_vt", bufs=4, space="PSUM") as psv:
                # --- Q: own 800 positions, bias bq, -> bf16
                for lo, hi in NCHUNKS:
                    ps = psq.tile([128, 512], f32, tag="qkv")
                    for j in range(2):
                        nc.tensor.matmul(ps[:, :hi - lo], wqkvT[j][:, 0:128],
                                         xf[j][:, lo:hi],
                                         start=(j == 0), stop=(j == 1))
                    nc.vector.tensor_scalar_add(Q[:, lo:hi], ps[:, :hi - lo], bq)
                # --- K: all 1600 positions, no bias, -> bf16
                for c in range(4):
                    lo, hi = 400 * c, 400 * (c + 1)
                    ps = psq.tile([128, 512], f32, tag="qkv")
                    for j in range(2):
                        nc.tensor.matmul(ps[:, :400], wqkvT[j][:, 128:256],
                                         xf[j][:, lo:hi],
                                         start=(j == 0), stop=(j == 1))
                    nc.vector.tensor_copy(K[:, lo:hi], ps[:, :400])
                # --- V (no bias) for dw branch: own rows + halo -> vpad
                # own rows: chunks of 11 and 9 rows (440/360 cols)
                for t in range(2):  # v channel tile (= heads 2t..2t+1)
                    for (r0, r1) in ((0, 11), (11, 20)):
                        lo, hi = 40 * r0, 40 * r1
                        ps = psq.tile([128, 512], f32, tag="qkv")
                        for j in range(2):
                            nc.tensor.matmul(
                                ps[:, :hi - lo],
                                wqkvT[j][:, 256 + 128 * t:256 + 128 * (t + 1)],
                                xf[j][:, lo:hi],
                                start=(j == 0), stop=(j == 1))
                        nc.vector.tensor_copy(
                            vpad[t][:, 1 + r0:1 + r1, 1:41],
                            ps[:, :hi - lo].rearrange("p (r c) -> p r c", c=40))
                    # halo rows (top=xh[:,0:40] -> row 0, bottom -> row 21)
                    ps = psq.tile([128, 512], f32, tag="qkv")
                    for j in range(2):
                        nc.tensor.matmul(
                            ps[:, :80],
                            wqkvT[j][:, 256 + 128 * t:256 + 128 * (t + 1)],
                            xh[j],
                            start=(j == 0), stop=(j == 1))
                    nc.vector.tensor_copy(vpad[t][:, 0:1, 1:41], ps[:, 0:40])
                    nc.vector.tensor_copy(vpad[t][:, 21:22, 1:41], ps[:, 40:80])

                # ===== Phase 2: v^T via direct matmul (same pool scope) ====
                SKIP_2 = max_phase < 2 or "2" in skip_set
                # vT[m,h,d] = sum_ic xf[ic,m] * Wv[ic,h*64+d]; ones col appended
                for i in range(MB if not SKIP_2 else 0):
                    mb = _mbs(i)
                    ps = psv.tile([128, 256], f32, tag="vt")
                    for j in range(2):
                        nc.tensor.matmul(ps[:mb, :],
                                         xf[j][:, 128 * i:128 * i + mb],
                                         wqkvT[j][:, 256:512],
                                         start=(j == 0), stop=(j == 1))
                    nc.vector.tensor_copy(
                        vT[:mb, i, :, 0:DH],
                        ps[:mb, :].rearrange("p (h d) -> p h d", h=NH))
                    nc.vector.memset(vT[:mb, i, :, DH:DH + 1], 1.0)

            # ================= Phase 3: depthwise 3x3 via diag matmuls ========
            SKIP_3 = max_phase < 3 or "3" in skip_set
            with tc.tile_pool(name="ps_dw", bufs=6, space="PSUM") as psd:
                for t in range(2 if not SKIP_3 else 0):
                    for ci, (r0, r1) in enumerate(((0, 12), (12, 20))):
                        nr = r1 - r0
                        ps = psd.tile([128, 512], f32, tag="dw")
                        first = True
                        for ky in range(3):
                            for kx in range(3):
                                tap = ky * 3 + kx
                                nc.tensor.matmul(
                                    ps[:, :nr * 40],
                                    wdiag[:, 9 * t + tap, :],
                                    vpad[t][:, r0 + ky:r1 + ky, kx:kx + 40],
                                    start=first, stop=False)
                                first = False
                        # bias / edge-mask correction (rank-10)
                        nc.tensor.matmul(ps[:, :nr * 40],
                                         wdwb[:, 128 * t:128 * (t + 1)],
                                         maug[:, 40 * r0:40 * r1],
                                         start=False, stop=True)
                        nc.vector.tensor_copy(dw_sb[t][:, 40 * r0:40 * r1],
                                              ps[:, :nr * 40])

            # ================= Phase 4: attention =================
            SKIP_4 = max_phase < 4 or "4" in skip_set
            # h-inner structure: per m-block, QK for all 4 heads row-group
            # packed (tile_position=(32h,0)), one exp op over [mb, 4, 400],
            # AV accumulated per head into its own 1-bank psum. AV for block
            # i is emitted after QK/exp of block i+1 so the scheduler keeps
            # ACT fed (pa is single-buffered: 4 banks + 4 pav banks = 8).
            with tc.tile_pool(name="ps_attn", bufs=1, space="PSUM") as psa, \
                 tc.tile_pool(name="ps_av", bufs=1, space="PSUM") as psav, \
                 tc.tile_pool(name="exps", bufs=5) as exps, \
                 tc.tile_pool(name="norm", bufs=2) as normp:
                for nh in ((0, 400), (400, 800)) if not SKIP_4 else ():
                    lo, hi = nh
                    nw = hi - lo
                    pavs = [psav.tile([128, 512], f32, tag=f"pav{h}",
                                      name=f"pav{h}_{lo}")
                            for h in range(NH)]
                    pending = None
                    for i in range(MB):
                        mb = _mbs(i)
                        pa = psa.tile([128, NH, 512], f32, tag="pa")
                        et = exps.tile([128, NH, 512], bf16, tag="et")
                        for h in range(NH):
                            nc.tensor.matmul(
                                pa[:mb, h, 0:nw],
                                K[32 * h:32 * h + 32, 128 * i:128 * i + mb],
                                Q[32 * h:32 * h + 32, lo:hi],
                                start=True, stop=True,
                                tile_position=(32 * h, 0))
                        nc.scalar.activation(et[:mb, :, 0:nw], pa[:mb, :, 0:nw],
                                             mybir.ActivationFunctionType.Exp,
                                             scale=SCALE)
                        if pending is not None:
                            _emit_av(nc, pavs, vT, pending[0], pending[1],
                                     nw, NH, DH, MB)
                        pending = (i, et)
                    _emit_av(nc, pavs, vT, pending[0], pending[1], nw, NH, DH,
                             MB)
                    # normalize each head for this n-half
                    for h in range(NH):
                        rs = normp.tile([65, 512], f32, tag="rs")
                        rep = normp.tile([64, 512], f32, tag="rep")
                        nc.vector.reciprocal(rs[64:65, 0:nw],
                                             pavs[h][64:65, 0:nw])
                        nc.tensor.matmul(pavs[h][64:128, 0:nw],
                                         onesrow[64:65, :], rs[64:65, 0:nw],
                                         start=True, stop=True,
                                         tile_position=(64, 64),
                                         skip_group_check=True)
                        nc.vector.tensor_copy(rep[:, 0:nw],
                                              pavs[h][64:128, 0:nw])
                        r0 = 64 * (h % 2)
                        nc.vector.tensor_mul(attn_out[h // 2][r0:r0 + 64,
                                                              lo:hi],
                                             pavs[h][0:64, 0:nw],
                                             rep[:, 0:nw])

            # ================= Phase 5: projection =================
            SKIP_5 = max_phase < 5 or "5" in skip_set
            with tc.tile_pool(name="ps_proj", bufs=2, space="PSUM") as psp:
                for t in range(2):
                    if SKIP_5:
                        nc.sync.dma_start(out=out_d[128 * t:128 * (t + 1), :],
                                          in_=out_sb[t])
                        continue
                    ps = psp.tile([128, 1024], f32, tag="proj")
                    for lo, hi in NCHUNKS:
                        for j in range(2):
                            nc.tensor.matmul(
                                ps[:, lo:hi],
                                wprojT[j][:, 128 * t:128 * (t + 1)],
                                attn_out[j][:, lo:hi],
                                start=(j == 0), stop=False)
                        for j in range(2):
                            nc.tensor.matmul(
                                ps[:, lo:hi],
                                wprojT[j][:, 128 * t:128 * (t + 1)],
                                dw_sb[j][:, lo:hi],
                                start=False, stop=(j == 1))
                    nc.vector.tensor_scalar_add(out_sb[t], ps[:, 0:NQ], bproj[t])
                    nc.sync.dma_start(out=out_d[128 * t:128 * (t + 1), :],
                                      in_=out_sb[t])

    nc.finalize()
    return nc


def _prep_host(inputs):
    """Fold BN into weights, build per-core in_maps."""
    x = np.asarray(inputs["x"], np.float32)

    def fold(g, b, m, v):
        s = np.asarray(g, np.float32) / np.sqrt(np.asarray(v, np.float32) + EPS)
        return s, np.asarray(b, np.float32) - np.asarray(m, np.float32) * s

    s_qkv, b_qkv = fold(inputs["g_qkv"], inputs["b_qkv"], inputs["m_qkv"],
                        inputs["v_qkv"])
    Wq = np.asarray(inputs["w_qkv"], np.float32)[:, :, 0, 0] * s_qkv[:, None]

    q_rows = np.concatenate([np.arange(h * 128, h * 128 + DK) for h in range(NH)])
    k_rows = q_rows + DK
    v_rows = np.concatenate([np.arange(h * 128 + 2 * DK, (h + 1) * 128)
                             for h in range(NH)])
    perm = np.concatenate([q_rows, k_rows, v_rows])
    wqkvT = np.ascontiguousarray(Wq[perm].T)          # [256, 512]
    bias_q = b_qkv[q_rows].reshape(128, 1)
    bias_v = b_qkv[v_rows]                            # [256] original ch order

    s_dw, b_dw = fold(inputs["g_dw"], inputs["b_dw"], inputs["m_dw"],
                      inputs["v_dw"])
    wdw = np.asarray(inputs["w_dw"], np.float32)[:, 0].reshape(CH, 9) * \
        s_dw[:, None]                                  # [256, 9]
    # diag matrices [128, 18, 128]: index t*... = ictile j, tap:
    wdiag = np.zeros((128, 2 * 9, 128), np.float32)
    for j in range(2):
        for tap in range(9):
            np.fill_diagonal(wdiag[:, 9 * j + tap, :], wdw[128 * j:128 * (j + 1), tap])
    wdiag = wdiag.reshape(128, 18 * 128)
    # rank-10 bias/edge correction: wdwb [10, 256]
    wdwb = np.zeros((10, CH), np.float32)
    wdwb[:9] = (wdw * bias_v[:, None]).T
    wdwb[9] = b_dw + bias_v

    s_pr, b_pr = fold(inputs["g_proj"], inputs["b_proj"], inputs["m_proj"],
                      inputs["v_proj"])
    wprojT = np.ascontiguousarray(
        (np.asarray(inputs["w_proj"], np.float32)[:, :, 0, 0] * s_pr[:, None]).T)
    bproj = b_pr.reshape(CH, 1)

    in_maps = []
    for core in range(NCORES):
        b, s = divmod(core, 2)
        xb = x[b].reshape(CH, HW)
        own = xb[:, s * NQ:(s + 1) * NQ]
        other = xb[:, (1 - s) * NQ:(2 - s) * NQ]
        xf = np.ascontiguousarray(np.concatenate([own, other], axis=1))
        xh = np.zeros((CH, 80), np.float32)
        if s == 0:
            xh[:, 40:80] = xb[:, 800:840]     # bottom halo = global row 20
        else:
            xh[:, 0:40] = xb[:, 760:800]      # top halo = global row 19
        # tap in-bounds mask over own 800 positions
        maug = np.zeros((10, NQ), np.float32)
        gr = s * 20 + np.arange(20)[:, None] + np.zeros((1, 40), int)   # [20,40]
        gc = np.zeros((20, 1), int) + np.arange(40)[None, :]
        for ky in range(3):
            for kx in range(3):
                inb = ((gr + ky - 1 >= 0) & (gr + ky - 1 <= 39) &
                       (gc + kx - 1 >= 0) & (gc + kx - 1 <= 39))
                maug[ky * 3 + kx] = inb.reshape(NQ).astype(np.float32)
        maug[9] = 1.0
        in_maps.append({
            "xf": xf.astype(BF16), "xh": xh.astype(BF16),
            "wqkvT": wqkvT.astype(BF16), "bq": bias_q,
            "wdiag": wdiag.astype(BF16), "wdwb": wdwb.astype(BF16),
            "maug": maug.astype(BF16),
            "wprojT": wprojT.astype(BF16), "bproj": bproj,
        })
    return in_maps


def kernel(**inputs):
    global _COMPILED, LAST_EXEC_NS, LAST_RESULTS
    from concourse import bass_utils

    if _COMPILED is None:
        _COMPILED = build_program()
    nc = _COMPILED
    in_maps = _prep_host(inputs)
    # trace=True needs antenv.axon_hooks (absent in this container)
    res = bass_utils.run_bass_kernel_spmd(
        nc, in_maps, core_ids=list(range(NCORES)), trace=False)
    LAST_EXEC_NS = res.exec_time_ns
    LAST_RESULTS = res
    y = np.zeros((B, CH, H, W), np.float32)
    for core in range(NCORES):
        b, s = divmod(core, 2)
        o = res.results[core]["out"]  # [256, 800]
        y[b, :, s * 20:(s + 1) * 20, :] = np.asarray(o, np.float32).reshape(
            CH, 20, 40)
    return y

